# revision 1
# baseline (speedup 1.0000x reference)
"""BitNet transformer block on 8 Trainium2 NeuronCores (Bass/Tile SPMD).

Sharding: data-parallel fold-balanced attention (core i owns query blocks
{i, 15-i} of 16 x 128 tokens; kv-proj token-sharded + AllGather), then
tensor-parallel MLP over INTER/8 with a token-chunked ReduceScatter.
Weights are ternary-quantized on host (exact {-1,0,+1} in bf16) with fp32
per-feature scales applied at PSUM evict. The residual after o_proj is
returned per-core and added during host-side unshard assembly.
"""

import sys

import numpy as np

try:
    import concourse.bass as bass  # noqa: F401
except Exception:  # pragma: no cover
    sys.path.insert(0, "/opt/trn_rl_repo")

import ml_dtypes
import concourse.bass as bass
import concourse.mybir as mybir
import concourse.tile as tile
from concourse import bacc
from concourse.bass_utils import run_bass_kernel_spmd

FP32 = mybir.dt.float32
BF16 = mybir.dt.bfloat16
BF = ml_dtypes.bfloat16

ALPHA = 0.7
EPS = 1e-5
NH = 16          # query heads
NKV = 4          # kv heads
D = 128          # head dim
H = 2048         # hidden
I_TOT = 8192     # mlp intermediate
S = 2048         # sequence
NC = 8           # cores
P = 128
HT = H // P      # 16 hidden tiles
B = S // P       # 16 token blocks
I_LOC = I_TOT // NC   # 1024 intermediate per core
IT = I_LOC // P       # 8 inter tiles per core
TOK = 256             # tokens owned per core (2 blocks)
NCHUNK = 4            # reduce-scatter token chunks
CH = S // NCHUNK      # 512 tokens per chunk

# gathered token order: core i contributes blocks [i, 15-i]
PERM = []
for _i in range(NC):
    PERM += [_i, 15 - _i]
# MLP/RS token order: lo blocks 0..7 then hi blocks 15..8
PERM_DOWN = list(range(8)) + list(range(15, 7, -1))

_CACHE = {}


def _build_program():
    nc = bacc.Bacc("TRN2", target_bir_lowering=False, debug=False, num_devices=NC)
    AF = mybir.ActivationFunctionType
    ALU = mybir.AluOpType
    rg = [list(range(NC))]
    CH = 256          # MLP token chunk (= one rank's tokens)
    NCH = 8
    HH = HT // 2

    # ---------------- inputs ----------------
    def dram_in(name, shape, dt=FP32):
        return nc.dram_tensor(name, shape, dt, kind="ExternalInput")

    xT_f = dram_in("xT_f", [P, HT, S], BF16)          # bf16 x^T ALL tokens (ln1 only)
    xT_own = dram_in("xT_own", [P, HT, TOK])          # fp32 x^T own cols (residual)
    cos_f = dram_in("cos_f", [P, S])
    sin_f = dram_in("sin_f", [P, S])
    wq_in = dram_in("wq", [P, 2, HT, P], BF16)        # my 2 heads [p, f, kt, c]
    wk_in = dram_in("wk", [P, HT, P], BF16)           # my kv head
    wv_in = dram_in("wv", [P, HT, P], BF16)           # my kv head (lhsT like k)
    wo_in = dram_in("wo", [HT, P, HT, P], BF16)
    wg_in = dram_in("wg", [IT, P, HT, P], BF16)
    wu_in = dram_in("wu", [IT, P, HT, P], BF16)
    wd_in = dram_in("wd", [P, IT, H], BF16)           # [p, it, ho]
    aq_in = dram_in("aq", [P, 2])
    ak_in = dram_in("ak", [P, 1])
    av_in = dram_in("av", [P, 1])
    ao_in = dram_in("ao", [P, HT])
    ag_in = dram_in("ag", [P, IT])
    au_in = dram_in("au", [P, IT])
    ad_in = dram_in("ad", [P, HT])
    rT_in = dram_in("rT", [P, P], BF16)               # rope rotate-half perm^T
    tril_in = dram_in("tril2", [P, TOK], BF16)        # [k, q] keep k<=q, both heads
    iden_in = dram_in("iden", [P, P], BF16)           # identity for PE transpose
    ones_f_in = dram_in("ones_f", [P, P])             # fp32 ones
    ones_b_in = dram_in("ones_b", [P, 1], BF16)       # bf16 ones column
    eps_in = dram_in("epsv", [P, 1])

    outT = nc.dram_tensor("outT", [TOK, H], FP32, kind="ExternalOutput")
    xmidT = nc.dram_tensor("xmidT", [P, HT, TOK], FP32, kind="ExternalOutput")

    a2a_lo_in = nc.dram_tensor("a2a_lo_in", [NC, P, 2, P], BF16)
    a2a_lo_out = nc.dram_tensor("a2a_lo_out", [NC, P, 2, P], BF16)
    a2a_hi_in = nc.dram_tensor("a2a_hi_in", [NC, P, 2, P], BF16)
    a2a_hi_out = nc.dram_tensor("a2a_hi_out", [NC, P, 2, P], BF16)
    h2_in_lo = nc.dram_tensor("h2_in_lo", [P, HT, P], BF16)
    h2_in_hi = nc.dram_tensor("h2_in_hi", [P, HT, P], BF16)
    h2_glo = nc.dram_tensor("h2_glo", [NC * P, HT, P], BF16, addr_space="Shared")
    h2_ghi = nc.dram_tensor("h2_ghi", [NC * P, HT, P], BF16, addr_space="Shared")

    with tile.TileContext(nc) as tc:
        const = tc.alloc_tile_pool(name="const", bufs=1)
        ones_f = const.tile([P, P], FP32)
        ones_b = const.tile([P, 1], BF16)
        rT = const.tile([P, P], BF16)
        iden = const.tile([P, P], BF16)
        tril2 = const.tile([P, TOK], BF16)
        aq = const.tile([P, 2], FP32)
        ak = const.tile([P, 1], FP32)
        av = const.tile([P, 1], FP32)
        ao = const.tile([P, HT], FP32)
        ag = const.tile([P, IT], FP32)
        au = const.tile([P, IT], FP32)
        ad = const.tile([P, HT], FP32)
        eps_t = const.tile([P, 1], FP32)
        for dst, src in [(ones_f, ones_f_in), (ones_b, ones_b_in), (rT, rT_in),
                         (eps_t, eps_in),
                         (iden, iden_in), (tril2, tril_in),
                         (aq, aq_in), (ak, ak_in), (av, av_in), (ao, ao_in),
                         (ag, ag_in), (au, au_in), (ad, ad_in)]:
            nc.sync.dma_start(dst[:], src[:])

        midpool = tc.alloc_tile_pool(name="midpool", bufs=1)
        x_mid = midpool.tile([P, HT, TOK], FP32)
        h2 = midpool.tile([P, HT, TOK], BF16)
        xopool = tc.alloc_tile_pool(name="xopool", bufs=1)
        xo = xopool.tile([P, HT, TOK], FP32)
        omypool = tc.alloc_tile_pool(name="omypool", bufs=1)
        o_my = omypool.tile([P, HT, TOK], BF16)      # post-A2A: 16 heads x my toks
        qkvpool = tc.alloc_tile_pool(name="qkvpool", bufs=1)
        q_my = qkvpool.tile([P, 2, S], BF16)         # my 2 heads, all tokens
        k_my = qkvpool.tile([P, B, P], BF16)         # my kv head [d, blk, tok]
        v_my = qkvpool.tile([P, B, P], BF16)         # my kv head [tok, blk, d]

        def rmsnorm_t(src3d, out3d, nt, psp, tmp):
            """[P,HT,nt] fp32 -> bf16 rmsnorm (partition reduce via ones-mm)"""
            ssq = psp.tile([1, 512], FP32, name="ssq")[:, :nt]
            for kt in range(HT):
                sqv = tmp.tile([P, 512], BF16, name="sqv")[:, :nt]
                nc.vector.tensor_mul(sqv[:], src3d[:, kt, :], src3d[:, kt, :])
                nc.tensor.matmul(ssq[:], ones_b[:], sqv[:],
                                 start=(kt == 0), stop=(kt == HT - 1))
            ms = tmp.tile([1, 512], FP32, name="ms")[:, :nt]
            nc.scalar.activation(ms[:], ssq[:], AF.Identity, bias=eps_t[0:1, :],
                                 scale=1.0 / H)
            rec = tmp.tile([1, 512], FP32, name="rec")[:, :nt]
            nc.vector.reciprocal(rec[:], ms[:])
            rsq = tmp.tile([1, 512], FP32, name="rsq")[:, :nt]
            nc.scalar.activation(rsq[:], rec[:], AF.Sqrt)
            bc = psp.tile([P, 512], FP32, name="bc")[:, :nt]
            nc.tensor.matmul(bc[:], ones_f[0:1, :], rsq[:], start=True, stop=True)
            for kt in range(HT):
                nc.vector.tensor_mul(out3d[:, kt, :], src3d[:, kt, :], bc[:])

        # ====== phase 1: ln1 (all tokens, chunked) + q/k/v TP projections ======
        CH4 = 512
        with tc.tile_pool(name="xc_pool", bufs=2) as xcp, \
             tc.tile_pool(name="hc_pool", bufs=2) as hcp, \
             tc.tile_pool(name="p1sb", bufs=2) as p1sb, \
             tc.tile_pool(name="p1ps", bufs=1, space="PSUM") as p1ps, \
             tc.tile_pool(name="wpool", bufs=2) as wp, \
             tc.tile_pool(name="p2ps", bufs=2, space="PSUM") as p2ps, \
             tc.tile_pool(name="rot_ps", bufs=2, space="PSUM") as rot_ps, \
             tc.tile_pool(name="vt_ps", bufs=2, space="PSUM") as vt_ps, \
             tc.tile_pool(name="p2sb", bufs=2) as p2sb, \
             tc.tile_pool(name="cs_pool", bufs=2) as csp, \
             tc.tile_pool(name="wres", bufs=1) as wres:
            wq_sb = wres.tile([P, 2, HT, P], BF16)
            nc.sync.dma_start(wq_sb[:], wq_in[:])
            wk_sb = wres.tile([P, HT, P], BF16)
            nc.sync.dma_start(wk_sb[:], wk_in[:])
            wv_sb = wres.tile([P, HT, P], BF16)
            nc.sync.dma_start(wv_sb[:], wv_in[:])
            for c4 in range(4):
                tsl = slice(c4 * CH4, (c4 + 1) * CH4)
                xc = xcp.tile([P, HT, CH4], BF16, name="xc")
                nc.scalar.dma_start(xc[:], xT_f[:, :, tsl])
                cfc = csp.tile([P, CH4], FP32, name="cfc")
                nc.scalar.dma_start(cfc[:], cos_f[:, tsl])
                sfc = csp.tile([P, CH4], FP32, name="sfc")
                nc.scalar.dma_start(sfc[:], sin_f[:, tsl])
                hc = hcp.tile([P, HT, CH4], BF16, name="hc")
                rmsnorm_t(xc, hc, CH4, p1ps, p1sb)
                # q: my 2 heads
                for f in range(2):
                    ps = p2ps.tile([P, CH4], FP32, name="pps")
                    for kt in range(HT):
                        nc.tensor.matmul(ps[:], wq_sb[:, f, kt, :], hc[:, kt, :],
                                         start=(kt == 0), stop=(kt == HT - 1))
                    qs = p2sb.tile([P, CH4], BF16, name="qs")
                    nc.vector.tensor_scalar_mul(qs[:], ps[:], aq[:, f:f + 1])
                    rot = rot_ps.tile([P, CH4], FP32, name="rot")
                    nc.tensor.matmul(rot[:], rT[:], qs[:], start=True, stop=True)
                    t1 = p2sb.tile([P, CH4], FP32, name="t1")
                    nc.vector.tensor_mul(t1[:], rot[:], sfc[:])
                    t2 = p2sb.tile([P, CH4], FP32, name="t2")
                    nc.vector.tensor_mul(t2[:], qs[:], cfc[:])
                    nc.vector.tensor_add(q_my[:, f, tsl], t1[:], t2[:])
                # k: my kv head
                ps = p2ps.tile([P, CH4], FP32, name="pps")
                for kt in range(HT):
                    nc.tensor.matmul(ps[:], wk_sb[:, kt, :], hc[:, kt, :],
                                     start=(kt == 0), stop=(kt == HT - 1))
                ks = p2sb.tile([P, CH4], BF16, name="qs")
                nc.vector.tensor_scalar_mul(ks[:], ps[:], ak[:, 0:1])
                rot = rot_ps.tile([P, CH4], FP32, name="rot")
                nc.tensor.matmul(rot[:], rT[:], ks[:], start=True, stop=True)
                t1 = p2sb.tile([P, CH4], FP32, name="t1")
                nc.vector.tensor_mul(t1[:], rot[:], sfc[:])
                t2 = p2sb.tile([P, CH4], FP32, name="t2")
                nc.vector.tensor_mul(t2[:], ks[:], cfc[:])
                nc.vector.tensor_add(
                    k_my[:, 4 * c4:4 * c4 + 4, :].rearrange("p b t -> p (b t)"),
                    t1[:], t2[:])
                # v: my kv head, then PE-transpose to [tok, d]
                ps = p2ps.tile([P, CH4], FP32, name="pps")
                for kt in range(HT):
                    nc.tensor.matmul(ps[:], wv_sb[:, kt, :], hc[:, kt, :],
                                     start=(kt == 0), stop=(kt == HT - 1))
                vtv = p2sb.tile([P, CH4], BF16, name="vtv")
                nc.vector.tensor_scalar_mul(vtv[:], ps[:], av[:, 0:1])
                for j in range(4):
                    vtp = vt_ps.tile([P, P], BF16, name="vtp")
                    nc.tensor.transpose(vtp[:], vtv[:, j * P:(j + 1) * P], iden[:])
                    nc.vector.tensor_copy(v_my[:, 4 * c4 + j, :], vtp[:])

        # ============= phase 2: attention (triangle, paired heads) =============
        with tc.tile_pool(name="a_ps", bufs=3, space="PSUM") as a_ps, \
             tc.tile_pool(name="o_ps", bufs=2, space="PSUM") as o_ps, \
             tc.tile_pool(name="l_ps", bufs=2, space="PSUM") as l_ps, \
             tc.tile_pool(name="bc_ps", bufs=1, space="PSUM") as bc_ps, \
             tc.tile_pool(name="a_sb", bufs=3) as a_sb:
            for qb in range(B):
                r_dst = min(qb, 15 - qb)
                off = 0 if qb < 8 else P
                ops = o_ps.tile([P, TOK], FP32, name="ops")
                lps = l_ps.tile([1, TOK], FP32, name="lps")
                qv = q_my[:, :, qb * P:(qb + 1) * P]    # [P, 2, 128]
                for kb in range(qb + 1):
                    sps = a_ps.tile([P, TOK], FP32, name="sps")
                    nc.tensor.matmul(sps[:], k_my[:, kb, :], qv,
                                     start=True, stop=True)
                    pm = a_sb.tile([P, TOK], BF16, name="pm")
                    nc.scalar.activation(pm[:], sps[:], AF.Exp)
                    if kb == qb:
                        pmm = a_sb.tile([P, TOK], BF16, name="pmm")
                        nc.vector.tensor_mul(pmm[:], pm[:], tril2[:])
                        pm = pmm
                    nc.tensor.matmul(lps[:], ones_b[:], pm[:],
                                     start=(kb == 0), stop=(kb == qb))
                    nc.tensor.matmul(ops[:], v_my[:, kb, :], pm[:],
                                     start=(kb == 0), stop=(kb == qb))
                lsb = a_sb.tile([1, TOK], FP32, name="lsb")
                nc.scalar.activation(lsb[:], lps[:], AF.Copy)
                linv = a_sb.tile([1, TOK], FP32, name="linv")
                nc.vector.reciprocal(linv[:], lsb[:])
                bca = bc_ps.tile([P, TOK], FP32, name="bca")
                nc.tensor.matmul(bca[:], ones_f[0:1, :], linv[:], start=True, stop=True)
                bcs = a_sb.tile([P, TOK], FP32, name="bcs")
                nc.scalar.activation(bcs[:], bca[:], AF.Copy)
                osb = a_sb.tile([P, TOK], BF16, name="osb")
                nc.vector.tensor_mul(osb[:], ops[:], bcs[:])
                dst = a2a_lo_in if qb < 8 else a2a_hi_in
                nc.sync.dma_start(
                    dst[r_dst][:],
                    osb[:].rearrange("p (h t) -> p h t", h=2))
                if qb == 7:
                    nc.gpsimd.collective_compute(
                        "AllToAll", ALU.bypass, ins=[a2a_lo_in[:]],
                        outs=[a2a_lo_out[:]], replica_groups=rg)
            nc.gpsimd.collective_compute(
                "AllToAll", ALU.bypass, ins=[a2a_hi_in[:]],
                outs=[a2a_hi_out[:]], replica_groups=rg)
        qkvpool.release()

        # ============= phase 3: o_proj + residual + ln2 (token halves) =============
        with tc.tile_pool(name="wo_pool", bufs=3) as wop, \
             tc.tile_pool(name="wo_res", bufs=1) as wores, \
             tc.tile_pool(name="p5ps", bufs=2, space="PSUM") as p5ps, \
             tc.tile_pool(name="p5sb", bufs=3) as p5sb:
            nc.sync.dma_start(xo[:], xT_own[:])
            wo_all = wores.tile([P, HT, HT, P], BF16)
            for f in range(HT):
                nc.scalar.dma_start(wo_all[:, f, :, :], wo_in[f])
            for j in range(NC):
                nc.sync.dma_start(o_my[:, 2 * j:2 * j + 2, 0:P], a2a_lo_out[j])
            for half, (h2_in, h2_g) in enumerate(
                    ((h2_in_lo, h2_glo), (h2_in_hi, h2_ghi))):
                csl = slice(half * P, (half + 1) * P)
                if half == 1:
                    for j in range(NC):
                        nc.sync.dma_start(o_my[:, 2 * j:2 * j + 2, P:TOK],
                                          a2a_hi_out[j])
                for f in range(HT):
                    ps = p5ps.tile([P, P], FP32, name="ops5")
                    for kt in range(HT):
                        nc.tensor.matmul(ps[:], wo_all[:, f, kt, :], o_my[:, kt, csl],
                                         start=(kt == 0), stop=(kt == HT - 1))
                    nc.vector.scalar_tensor_tensor(
                        x_mid[:, f, csl], ps[:], ao[:, f:f + 1],
                        xo[:, f, csl], ALU.mult, ALU.add)
                h2h = p5sb.tile([P, HT, P], BF16, name="h2h", tag="h2h")
                rmsnorm_t(x_mid[:, :, csl], h2h, P, p5ps, p5sb)
                nc.sync.dma_start(h2_in[:], h2h[:])
                nc.gpsimd.collective_compute(
                    "AllGather", ALU.bypass, ins=[h2_in[:]],
                    outs=[h2_g[:]], replica_groups=rg)
            nc.sync.dma_start(xmidT[:], x_mid[:])
        omypool.release()
        xopool.release()
        midpool.release()
        h2lov = h2_glo[:].rearrange("(r p) kt t -> r p kt t", r=NC)
        h2hiv = h2_ghi[:].rearrange("(r p) kt t -> r p kt t", r=NC)

        # ============= phase 5: MLP (TP over inter) + RS =============
        CHM = 512
        with tc.tile_pool(name="wd_res", bufs=1) as wdres, \
             tc.tile_pool(name="h2c_pool", bufs=2) as h2cp, \
             tc.tile_pool(name="m_pool", bufs=2) as mp, \
             tc.tile_pool(name="wgu_pool", bufs=3) as wgup, \
             tc.tile_pool(name="p7ps", bufs=2, space="PSUM") as p7ps, \
             tc.tile_pool(name="p7sb", bufs=3) as p7sb:
            wd_sb = wdres.tile([P, IT, H], BF16)
            nc.scalar.dma_start(wd_sb[:], wd_in[:])
            for c in range(4):
                h2v = h2lov if c < 2 else h2hiv
                rbase = (c % 2) * 4
                h2c = h2cp.tile([P, HT, CHM], BF16, name="h2c")
                for j in range(4):
                    nc.scalar.dma_start(h2c[:, :, j * P:(j + 1) * P],
                                        h2v[rbase + j])
                m_all = mp.tile([P, IT, CHM], BF16, name="m_all")
                for f in range(IT):
                    wtg = wgup.tile([P, HT, P], BF16, name="wtg")
                    nc.scalar.dma_start(wtg[:], wg_in[f])
                    gps = p7ps.tile([P, CHM], FP32, name="gps")
                    for kt in range(HT):
                        nc.tensor.matmul(gps[:], wtg[:, kt, :], h2c[:, kt, :],
                                         start=(kt == 0), stop=(kt == HT - 1))
                    wtu = wgup.tile([P, HT, P], BF16, name="wtu")
                    nc.scalar.dma_start(wtu[:], wu_in[f])
                    ups = p7ps.tile([P, CHM], FP32, name="ups")
                    for kt in range(HT):
                        nc.tensor.matmul(ups[:], wtu[:, kt, :], h2c[:, kt, :],
                                         start=(kt == 0), stop=(kt == HT - 1))
                    gr = p7sb.tile([P, CHM], FP32, name="gr")
                    nc.vector.tensor_scalar(gr[:], gps[:], ag[:, f:f + 1], 0.0,
                                            ALU.mult, ALU.max)
                    g2 = p7sb.tile([P, CHM], FP32, name="g2")
                    nc.vector.tensor_mul(g2[:], gr[:], gr[:])
                    nc.vector.scalar_tensor_tensor(m_all[:, f, :], ups[:],
                                                   au[:, f:f + 1], g2[:],
                                                   ALU.mult, ALU.mult)
                rs_a = nc.dram_tensor(f"rs_in_{c}a", [H, TOK], FP32)
                rs_b = nc.dram_tensor(f"rs_in_{c}b", [H, TOK], FP32)
                rs_iva = rs_a[:].rearrange("(f p) t -> f p t", p=P)
                rs_ivb = rs_b[:].rearrange("(f p) t -> f p t", p=P)
                if c < 3:
                    for f in range(HT):
                        dps = p7ps.tile([P, CHM], FP32, name="dps")
                        for it in range(IT):
                            nc.tensor.matmul(dps[:], wd_sb[:, it, f * P:(f + 1) * P],
                                             m_all[:, it, :],
                                             start=(it == 0), stop=(it == IT - 1))
                        dn = p7sb.tile([P, CHM], FP32, name="dn")
                        nc.vector.tensor_scalar_mul(dn[:], dps[:], ad[:, f:f + 1])
                        nc.sync.dma_start(rs_iva[f], dn[:, 0:TOK])
                        nc.sync.dma_start(rs_ivb[f], dn[:, TOK:CHM])
                    for hf, rs_in in enumerate((rs_a, rs_b)):
                        rs_out = nc.dram_tensor(f"rso_{c}{hf}", [TOK, TOK], FP32)
                        nc.gpsimd.collective_compute(
                            "ReduceScatter", ALU.add, ins=[rs_in[:]],
                            outs=[rs_out[:]], replica_groups=rg)
                        nc.sync.dma_start(
                            outT[:, (2 * c + hf) * TOK:(2 * c + hf + 1) * TOK],
                            rs_out[:])
                else:
                    # last chunk: split down by token halves so RS pieces overlap
                    for hf, (rs_in, rs_iv) in enumerate(
                            ((rs_a, rs_iva), (rs_b, rs_ivb))):
                        tsl2 = slice(hf * TOK, (hf + 1) * TOK)
                        for f in range(HT):
                            dps = p7ps.tile([P, CHM], FP32, name="dps")[:, 0:TOK]
                            for it in range(IT):
                                nc.tensor.matmul(
                                    dps[:], wd_sb[:, it, f * P:(f + 1) * P],
                                    m_all[:, it, tsl2],
                                    start=(it == 0), stop=(it == IT - 1))
                            dn = p7sb.tile([P, CHM], FP32, name="dn")[:, 0:TOK]
                            nc.vector.tensor_scalar_mul(dn[:], dps[:], ad[:, f:f + 1])
                            nc.sync.dma_start(rs_iv[f], dn[:])
                        rs_out = nc.dram_tensor(f"rso_3{hf}", [TOK, TOK], FP32)
                        nc.gpsimd.collective_compute(
                            "ReduceScatter", ALU.add, ins=[rs_in[:]],
                            outs=[rs_out[:]], replica_groups=rg)
                        nc.sync.dma_start(
                            outT[:, (6 + hf) * TOK:(7 + hf) * TOK], rs_out[:])

        const.release()

    nc.finalize()
    return nc


def _ternary(w, fold_row=None):
    """Quantize [O, Hin] fp32 -> (ternary fp32 {-1,0,1}, absmean [O])."""
    w = np.asarray(w, dtype=np.float32)
    am = np.mean(np.abs(w), axis=1)
    t = np.sign(w) * (np.abs(w) > ALPHA * am[:, None]).astype(np.float32)
    if fold_row is not None:
        t = t * fold_row[None, :]
    return t, am


def _wlhsT(tern, n_f):
    """ternary [O, Hin] -> lhsT input layout [f, p, kt, c] bf16 (tile (kt,f):
    rows Hin-chunk kt, cols O-chunk f)."""
    o, hin = tern.shape
    kt = hin // P
    assert n_f * P == o
    wT = np.ascontiguousarray(tern.T)  # [Hin, O]
    return np.ascontiguousarray(
        wT.reshape(kt, P, n_f, P).transpose(2, 1, 0, 3)).astype(BF)


def _scale_tiles(a):
    """[O] -> [P, O//P] with column f = features f*128..f*128+127."""
    return np.ascontiguousarray(a.reshape(-1, P).T).astype(np.float32)


def _pcol(x2d):
    """[K, T] -> [P, K//P, T] (partition-major for direct DMA)."""
    k, t = x2d.shape
    return np.ascontiguousarray(
        x2d.reshape(k // P, P, t).transpose(1, 0, 2)).astype(np.float32)


def kernel(x, cos, sin, wq, wk, wv, wo, wg, wu, wd, ln1_w, ln2_w):
    x = np.asarray(x, dtype=np.float32)
    b, s, hdim = x.shape
    assert (b, s, hdim) == (1, S, H)

    if "nc" not in _CACHE:
        _CACHE["nc"] = _build_program()
    nc = _CACHE["nc"]

    ln1 = np.asarray(ln1_w, dtype=np.float32)
    ln2 = np.asarray(ln2_w, dtype=np.float32)

    tq, amq = _ternary(wq, fold_row=ln1)
    tk, amk = _ternary(wk, fold_row=ln1)
    tv, amv = _ternary(wv, fold_row=ln1)
    to, amo = _ternary(wo)
    tg, amg = _ternary(wg, fold_row=ln2)
    tu, amu = _ternary(wu, fold_row=ln2)
    td, amd = _ternary(wd)

    wq_h = _wlhsT(tq, NH)        # [16, P, HT, P]
    wk_h = _wlhsT(tk, NKV)       # [4, P, HT, P]
    wv_h = _wlhsT(tv, NKV)
    wo_h = _wlhsT(to, HT)
    wg_h = _wlhsT(tg, I_TOT // P)
    wu_h = _wlhsT(tu, I_TOT // P)
    wd_h = np.ascontiguousarray(
        td.T.reshape(I_TOT // P, P, H).transpose(1, 0, 2)).astype(BF)  # [P,64,H]

    aq_h = _scale_tiles(amq / np.sqrt(np.float32(D)))
    ak_h = _scale_tiles(amk)
    av_h = _scale_tiles(amv)
    ao_h = _scale_tiles(amo)
    ag_h = _scale_tiles(amg)
    au_h = _scale_tiles(amu)
    ad_h = _scale_tiles(amd)

    x2 = x[0]
    xT = np.ascontiguousarray(x2.T)
    xT_f = _pcol(xT)
    cosT = np.ascontiguousarray(np.asarray(cos, np.float32)[0, 0].T)
    sinT = np.ascontiguousarray(np.asarray(sin, np.float32)[0, 0].T)

    R = np.zeros((P, P), np.float32)
    for m in range(64):
        R[m, m + 64] = -1.0
        R[m + 64, m] = 1.0
    rT_h = np.ascontiguousarray(R.T).astype(BF)
    ones_f = np.ones((P, P), np.float32)
    ones_b = np.ones((P, 1), np.float32).astype(BF)
    triu = np.triu(np.ones((P, P), np.float32))
    tril2_h = np.ascontiguousarray(np.concatenate([triu, triu], axis=1)).astype(BF)
    iden_h = np.eye(P, dtype=np.float32).astype(BF)

    in_maps = []
    for i in range(NC):
        blo, bhi = i, 15 - i
        own_cols = np.r_[blo * P:(blo + 1) * P, bhi * P:(bhi + 1) * P]
        kvh = i // 2
        islice = slice(i * IT, (i + 1) * IT)
        in_maps.append({
            "xT_f": xT_f.astype(BF),
            "xT_own": _pcol(xT[:, own_cols]),
            "cos_f": cosT, "sin_f": sinT,
            "wq": np.ascontiguousarray(wq_h[2 * i:2 * i + 2].transpose(1, 0, 2, 3)),
            "wk": np.ascontiguousarray(wk_h[kvh]),
            "wv": np.ascontiguousarray(wv_h[kvh]),
            "wo": wo_h,
            "wg": np.ascontiguousarray(wg_h[islice]),
            "wu": np.ascontiguousarray(wu_h[islice]),
            "wd": np.ascontiguousarray(wd_h[:, islice, :]),
            "aq": np.ascontiguousarray(aq_h[:, 2 * i:2 * i + 2]),
            "ak": np.ascontiguousarray(ak_h[:, kvh:kvh + 1]),
            "av": np.ascontiguousarray(av_h[:, kvh:kvh + 1]),
            "ao": ao_h,
            "ag": np.ascontiguousarray(ag_h[:, islice]),
            "au": np.ascontiguousarray(au_h[:, islice]),
            "ad": ad_h,
            "rT": rT_h, "tril2": tril2_h, "iden": iden_h,
            "ones_f": ones_f, "ones_b": ones_b,
            "epsv": np.full((P, 1), EPS, np.float32),
        })

    res = run_bass_kernel_spmd(nc, in_maps, list(range(NC)))
    _CACHE["last_result"] = res

    down_T = np.concatenate([res.results[i]["outT"] for i in range(NC)], axis=0)
    xmid_T = np.concatenate(
        [res.results[i]["xmidT"].transpose(1, 0, 2).reshape(H, TOK)
         for i in range(NC)], axis=1)
    out_T = np.empty_like(down_T)
    for j, blk in enumerate(PERM_DOWN):
        out_T[:, blk * P:(blk + 1) * P] = down_T[:, j * P:(j + 1) * P]
    for j, blk in enumerate(PERM):
        out_T[:, blk * P:(blk + 1) * P] += xmid_T[:, j * P:(j + 1) * P]
    return np.ascontiguousarray(out_T.T).reshape(1, S, H).astype(np.float32)


if __name__ == "__main__":
    nc = _build_program()
    print("build OK; instructions:",
          sum(len(b.instructions) for f in nc.m.functions for b in f.blocks))



# revision 7
# speedup vs baseline: 1.2233x; 1.2233x over previous
"""BitNet transformer block on 8 Trainium2 NeuronCores (Bass/Tile SPMD).

v2: fold-balanced head-parallel attention (core i owns heads {2i,2i+1},
query blocks fold-paired; A2A to token-parallel), then pair-wise TP-2
MLP (cores {2j,2j+1} split INTER 4096/4096 over their 512 tokens) with
fp8e4m3 DoubleRow matmuls for q/k/v/gate/up (ternary weights are exact
in fp8), bf16 down-proj, and chunked pair ReduceScatter in bf16.
The o_proj residual x_mid is returned per-core and added on the host
during unshard assembly (as in v1).
"""

import sys

import numpy as np

try:
    import concourse.bass as bass  # noqa: F401
except Exception:  # pragma: no cover
    sys.path.insert(0, "/opt/trn_rl_repo")

import ml_dtypes
import concourse.bass as bass
import concourse.mybir as mybir
import concourse.tile as tile
from concourse import bacc
from concourse.bass_utils import run_bass_kernel_spmd

FP32 = mybir.dt.float32
BF16 = mybir.dt.bfloat16
FP8 = mybir.dt.float8e4
BF = ml_dtypes.bfloat16
F8 = ml_dtypes.float8_e4m3

ALPHA = 0.7
EPS = 1e-5
NH = 16          # query heads
NKV = 4          # kv heads
D = 128          # head dim
H = 2048         # hidden
I_TOT = 8192     # mlp intermediate
S = 2048         # sequence
NC = 8           # cores
P = 128
HT = H // P      # 16 hidden tiles
HT2 = HT // 2    # 8 hidden tile-pairs (fp8 DoubleRow)
B = S // P       # 16 token blocks
I_LOC = I_TOT // 2    # 4096 intermediate per core (TP-2)
IT = I_LOC // P       # 32 inter tiles per core
TOK = 256             # tokens owned per core (2 blocks)
PTOK = 512            # tokens owned per pair
DR = mybir.MatmulPerfMode.DoubleRow

_CACHE = {}


def _build_program():
    nc = bacc.Bacc("TRN2", target_bir_lowering=False, debug=False, num_devices=NC)
    AF = mybir.ActivationFunctionType
    ALU = mybir.AluOpType
    rg_all = [list(range(NC))]
    rg_pair = [[2 * j, 2 * j + 1] for j in range(NC // 2)]

    # ---------------- inputs ----------------
    def dram_in(name, shape, dt=FP32):
        return nc.dram_tensor(name, shape, dt, kind="ExternalInput")

    xT_f = dram_in("xT_f", [P, HT, S], BF16)          # bf16 x^T ALL tokens (ln1 only)
    xT_own = dram_in("xT_own", [P, HT, TOK])          # fp32 x^T own cols (residual)
    cos_f = dram_in("cos_f", [P, S], BF16)
    sin_f = dram_in("sin_f", [P, S], BF16)
    wq_in = dram_in("wq", [P, 2, HT2, 2, P], FP8)     # my 2 heads [p,f,b,i,m]
    wk_in = dram_in("wk", [P, HT2, 2, P], FP8)        # my kv head
    wv_in = dram_in("wv", [P, HT2, 2, P], FP8)
    wo_in = dram_in("wo", [HT, P, HT, P], BF16)
    wg_in = dram_in("wg", [IT, P, HT2, 2, P], FP8)    # [f,p,b,i,m]
    wu_in = dram_in("wu", [IT, P, HT2, 2, P], FP8)
    wd_in = dram_in("wd", [HT, P, IT, P], BF16)       # [fo,p,it,m]
    aq_in = dram_in("aq", [P, 2])
    ak_in = dram_in("ak", [P, 1])
    av_in = dram_in("av", [P, 1])
    ao_in = dram_in("ao", [P, HT])
    ag_in = dram_in("ag", [P, IT])
    au_in = dram_in("au", [P, IT])
    ad_in = dram_in("ad", [P, HT])
    rT_in = dram_in("rT", [P, P], BF16)               # rope rotate-half perm^T
    tril_in = dram_in("tril2", [P, TOK], BF16)        # [k, q] keep k<=q, both heads
    iden_in = dram_in("iden", [P, P], BF16)           # identity for PE transpose
    ones_f_in = dram_in("ones_f", [P, P])             # fp32 ones
    ones_b_in = dram_in("ones_b", [P, 1], BF16)       # bf16 ones column
    invh_b_in = dram_in("invh_b", [P, 1], BF16)       # bf16 1/H column

    xmidT = nc.dram_tensor("xmidT", [P, HT, TOK], FP32, kind="ExternalOutput")
    outD = nc.dram_tensor("outD", [4, TOK, PTOK], BF16, kind="ExternalOutput")

    a2a_lo_in = nc.dram_tensor("a2a_lo_in", [NC, P, 2, P], BF16)
    a2a_lo_out = nc.dram_tensor("a2a_lo_out", [NC, P, 2, P], BF16)
    a2a_hi_in = nc.dram_tensor("a2a_hi_in", [NC, P, 2, P], BF16)
    a2a_hi_out = nc.dram_tensor("a2a_hi_out", [NC, P, 2, P], BF16)
    ag_lo_in = nc.dram_tensor("ag_lo_in", [P, HT, P], FP8)
    ag_hi_in = nc.dram_tensor("ag_hi_in", [P, HT, P], FP8)
    ag_lo_out = nc.dram_tensor("ag_lo_out", [2, P, HT, P], FP8)
    ag_hi_out = nc.dram_tensor("ag_hi_out", [2, P, HT, P], FP8)

    with tile.TileContext(nc) as tc:
        const = tc.alloc_tile_pool(name="const", bufs=1)
        ones_f = const.tile([P, P], FP32)
        ones_b = const.tile([P, 1], BF16)
        invh_b = const.tile([P, 1], BF16)
        rT = const.tile([P, P], BF16)
        iden = const.tile([P, P], BF16)
        tril2 = const.tile([P, TOK], BF16)
        aq = const.tile([P, 2], FP32)
        ak = const.tile([P, 1], FP32)
        av = const.tile([P, 1], FP32)
        ao = const.tile([P, HT], FP32)
        ag = const.tile([P, IT], FP32)
        au = const.tile([P, IT], FP32)
        ad = const.tile([P, HT], FP32)
        for dst, src in [(ones_f, ones_f_in), (ones_b, ones_b_in),
                         (invh_b, invh_b_in), (rT, rT_in),
                         (iden, iden_in), (tril2, tril_in),
                         (aq, aq_in), (ak, ak_in), (av, av_in), (ao, ao_in),
                         (ag, ag_in), (au, au_in), (ad, ad_in)]:
            nc.sync.dma_start(dst[:], src[:])

        midpool = tc.alloc_tile_pool(name="midpool", bufs=1)
        x_mid = midpool.tile([P, HT, TOK], FP32)
        xopool = tc.alloc_tile_pool(name="xopool", bufs=1)
        xo = xopool.tile([P, HT, TOK], FP32)
        omypool = tc.alloc_tile_pool(name="omypool", bufs=1)
        o_my = omypool.tile([P, HT, TOK], BF16)      # post-A2A: 16 heads x my toks
        qkvpool = tc.alloc_tile_pool(name="qkvpool", bufs=1)
        q_my = qkvpool.tile([P, 2, S], BF16)         # my 2 heads, all tokens
        k_my = qkvpool.tile([P, B, P], BF16)         # my kv head [d, blk, tok]
        v_my = qkvpool.tile([P, B, P], BF16)         # my kv head [tok, blk, d]

        def rmsnorm_t(src3d, out3d, nt, psp, tmp, odt):
            """[P,HT,nt] -> rmsnorm, partition reduce via (1/H)-column matmul,
            then broadcast-matmul BEFORE reciprocal so DVE runs on 128 lanes."""
            ssq = psp.tile([1, 512], FP32, name="ssq")[:, :nt]
            for kt in range(HT):
                sqv = tmp.tile([P, 512], BF16, name="sqv")[:, :nt]
                nc.vector.tensor_mul(sqv[:], src3d[:, kt, :], src3d[:, kt, :])
                nc.tensor.matmul(ssq[:], invh_b[:], sqv[:],
                                 start=(kt == 0), stop=(kt == HT - 1))
            ssb = tmp.tile([1, 512], FP32, name="ssb")[:, :nt]
            nc.scalar.activation(ssb[:], ssq[:], AF.Copy)
            msb = psp.tile([P, 512], FP32, name="msb")[:, :nt]
            nc.tensor.matmul(msb[:], ones_f[0:1, :], ssb[:], start=True, stop=True)
            rec = tmp.tile([P, 512], FP32, name="rec")[:, :nt]
            nc.vector.reciprocal(rec[:], msb[:])
            rsq = tmp.tile([P, 512], BF16, name="rsq")[:, :nt]
            nc.scalar.activation(rsq[:], rec[:], AF.Sqrt)
            for kt in range(HT):
                nc.vector.tensor_mul(out3d[:, kt, :], src3d[:, kt, :], rsq[:])
            _ = odt

        # ====== phase 1: ln1 (all tokens, chunked) + q/k/v TP projections ======
        CH4 = 512
        with tc.tile_pool(name="xc_pool", bufs=2) as xcp, \
             tc.tile_pool(name="hc_pool", bufs=2) as hcp, \
             tc.tile_pool(name="p1sb", bufs=2) as p1sb, \
             tc.tile_pool(name="p1ps", bufs=1, space="PSUM") as p1ps, \
             tc.tile_pool(name="p2ps", bufs=2, space="PSUM") as p2ps, \
             tc.tile_pool(name="rot_ps", bufs=2, space="PSUM") as rot_ps, \
             tc.tile_pool(name="vt_ps", bufs=2, space="PSUM") as vt_ps, \
             tc.tile_pool(name="p2sb", bufs=2) as p2sb, \
             tc.tile_pool(name="cs_pool", bufs=2) as csp, \
             tc.tile_pool(name="wres", bufs=1) as wres:
            wq_sb = wres.tile([P, 2, HT2, 2, P], FP8)
            nc.sync.dma_start(wq_sb[:], wq_in[:])
            wk_sb = wres.tile([P, HT2, 2, P], FP8)
            nc.sync.dma_start(wk_sb[:], wk_in[:])
            wv_sb = wres.tile([P, HT2, 2, P], FP8)
            nc.sync.dma_start(wv_sb[:], wv_in[:])
            for c4 in range(4):
                tsl = slice(c4 * CH4, (c4 + 1) * CH4)
                xc = xcp.tile([P, HT, CH4], BF16, name="xc")
                nc.scalar.dma_start(xc[:], xT_f[:, :, tsl])
                cfc = csp.tile([P, CH4], BF16, name="cfc")
                nc.scalar.dma_start(cfc[:], cos_f[:, tsl])
                sfc = csp.tile([P, CH4], BF16, name="sfc")
                nc.scalar.dma_start(sfc[:], sin_f[:, tsl])
                hc = hcp.tile([P, HT, CH4], FP8, name="hc")
                rmsnorm_t(xc, hc, CH4, p1ps, p1sb, FP8)
                # q: my 2 heads (fp8 DoubleRow over hidden pairs)
                for f in range(2):
                    ps = p2ps.tile([P, CH4], FP32, name="pps")
                    for b in range(HT2):
                        nc.tensor.matmul(ps[:], wq_sb[:, f, b, :, :],
                                         hc[:, 2 * b:2 * b + 2, :],
                                         start=(b == 0), stop=(b == HT2 - 1),
                                         perf_mode=DR)
                    qs = p2sb.tile([P, CH4], BF16, name="qs")
                    nc.scalar.activation(qs[:], ps[:], AF.Copy,
                                         scale=aq[:, f:f + 1])
                    rot = rot_ps.tile([P, CH4], FP32, name="rot")
                    nc.tensor.matmul(rot[:], rT[:], qs[:], start=True, stop=True)
                    t1 = p2sb.tile([P, CH4], BF16, name="t1")
                    nc.vector.tensor_mul(t1[:], rot[:], sfc[:])
                    t2 = p2sb.tile([P, CH4], BF16, name="t2")
                    nc.vector.tensor_mul(t2[:], qs[:], cfc[:])
                    nc.gpsimd.tensor_add(q_my[:, f, tsl], t1[:], t2[:])
                # k: my kv head
                ps = p2ps.tile([P, CH4], FP32, name="pps")
                for b in range(HT2):
                    nc.tensor.matmul(ps[:], wk_sb[:, b, :, :],
                                     hc[:, 2 * b:2 * b + 2, :],
                                     start=(b == 0), stop=(b == HT2 - 1),
                                     perf_mode=DR)
                ks = p2sb.tile([P, CH4], BF16, name="qs")
                nc.scalar.activation(ks[:], ps[:], AF.Copy, scale=ak[:, 0:1])
                rot = rot_ps.tile([P, CH4], FP32, name="rot")
                nc.tensor.matmul(rot[:], rT[:], ks[:], start=True, stop=True)
                t1 = p2sb.tile([P, CH4], BF16, name="t1")
                nc.vector.tensor_mul(t1[:], rot[:], sfc[:])
                t2 = p2sb.tile([P, CH4], BF16, name="t2")
                nc.vector.tensor_mul(t2[:], ks[:], cfc[:])
                nc.gpsimd.tensor_add(
                    k_my[:, 4 * c4:4 * c4 + 4, :].rearrange("p b t -> p (b t)"),
                    t1[:], t2[:])
                # v: my kv head, then PE-transpose to [tok, d]
                ps = p2ps.tile([P, CH4], FP32, name="pps")
                for b in range(HT2):
                    nc.tensor.matmul(ps[:], wv_sb[:, b, :, :],
                                     hc[:, 2 * b:2 * b + 2, :],
                                     start=(b == 0), stop=(b == HT2 - 1),
                                     perf_mode=DR)
                vtv = p2sb.tile([P, CH4], BF16, name="vtv")
                nc.scalar.activation(vtv[:], ps[:], AF.Copy, scale=av[:, 0:1])
                for j in range(4):
                    vtp = vt_ps.tile([P, P], BF16, name="vtp")
                    nc.tensor.transpose(vtp[:], vtv[:, j * P:(j + 1) * P], iden[:])
                    nc.vector.tensor_copy(v_my[:, 4 * c4 + j, :], vtp[:])

        # ============= phase 2: attention (triangle, paired heads) =============
        with tc.tile_pool(name="a_ps", bufs=3, space="PSUM") as a_ps, \
             tc.tile_pool(name="o_ps", bufs=2, space="PSUM") as o_ps, \
             tc.tile_pool(name="l_ps", bufs=2, space="PSUM") as l_ps, \
             tc.tile_pool(name="bc_ps", bufs=1, space="PSUM") as bc_ps, \
             tc.tile_pool(name="a_sb", bufs=3) as a_sb:
            for qb in range(B):
                r_dst = min(qb, 15 - qb)
                ops = o_ps.tile([P, TOK], FP32, name="ops")
                lps = l_ps.tile([1, TOK], FP32, name="lps")
                qv = q_my[:, :, qb * P:(qb + 1) * P]    # [P, 2, 128]
                for kb in range(qb + 1):
                    sps = a_ps.tile([P, TOK], FP32, name="sps")
                    nc.tensor.matmul(sps[:], k_my[:, kb, :], qv,
                                     start=True, stop=True)
                    pm = a_sb.tile([P, TOK], BF16, name="pm")
                    nc.scalar.activation(pm[:], sps[:], AF.Exp)
                    if kb == qb:
                        pmm = a_sb.tile([P, TOK], BF16, name="pmm")
                        nc.vector.tensor_mul(pmm[:], pm[:], tril2[:])
                        pm = pmm
                    nc.tensor.matmul(lps[:], ones_b[:], pm[:],
                                     start=(kb == 0), stop=(kb == qb))
                    nc.tensor.matmul(ops[:], v_my[:, kb, :], pm[:],
                                     start=(kb == 0), stop=(kb == qb))
                lsb = a_sb.tile([1, TOK], FP32, name="lsb")
                nc.scalar.activation(lsb[:], lps[:], AF.Copy)
                bca = bc_ps.tile([P, TOK], FP32, name="bca")
                nc.tensor.matmul(bca[:], ones_f[0:1, :], lsb[:], start=True, stop=True)
                linv = a_sb.tile([P, TOK], FP32, name="linv")
                nc.vector.reciprocal(linv[:], bca[:])
                osb = a_sb.tile([P, TOK], BF16, name="osb")
                nc.vector.tensor_mul(osb[:], ops[:], linv[:])
                dst = a2a_lo_in if qb < 8 else a2a_hi_in
                nc.sync.dma_start(
                    dst[r_dst][:],
                    osb[:].rearrange("p (h t) -> p h t", h=2))
                if qb == 7:
                    nc.gpsimd.collective_compute(
                        "AllToAll", ALU.bypass, ins=[a2a_lo_in[:]],
                        outs=[a2a_lo_out[:]], replica_groups=rg_all)
            nc.gpsimd.collective_compute(
                "AllToAll", ALU.bypass, ins=[a2a_hi_in[:]],
                outs=[a2a_hi_out[:]], replica_groups=rg_all)
        qkvpool.release()

        # ===== phase 3: o_proj + residual + ln2 (token halves) + pair-AG =====
        with tc.tile_pool(name="wo_pool", bufs=3) as wop, \
             tc.tile_pool(name="wo_res", bufs=1) as wores, \
             tc.tile_pool(name="p5ps", bufs=2, space="PSUM") as p5ps, \
             tc.tile_pool(name="p5sb", bufs=3) as p5sb:
            nc.sync.dma_start(xo[:], xT_own[:])
            wo_all = wores.tile([P, HT, HT, P], BF16)
            for f in range(HT):
                nc.scalar.dma_start(wo_all[:, f, :, :], wo_in[f])
            for j in range(NC):
                nc.sync.dma_start(o_my[:, 2 * j:2 * j + 2, 0:P], a2a_lo_out[j])
            for half, (ag_in_d, ag_out_d) in enumerate(
                    ((ag_lo_in, ag_lo_out), (ag_hi_in, ag_hi_out))):
                csl = slice(half * P, (half + 1) * P)
                if half == 1:
                    for j in range(NC):
                        nc.sync.dma_start(o_my[:, 2 * j:2 * j + 2, P:TOK],
                                          a2a_hi_out[j])
                for f in range(HT):
                    ps = p5ps.tile([P, P], FP32, name="ops5")
                    for kt in range(HT):
                        nc.tensor.matmul(ps[:], wo_all[:, f, kt, :], o_my[:, kt, csl],
                                         start=(kt == 0), stop=(kt == HT - 1))
                    nc.vector.scalar_tensor_tensor(
                        x_mid[:, f, csl], ps[:], ao[:, f:f + 1],
                        xo[:, f, csl], ALU.mult, ALU.add)
                h2h = p5sb.tile([P, HT, P], FP8, name="h2h", tag="h2h")
                rmsnorm_t(x_mid[:, :, csl], h2h, P, p5ps, p5sb, FP8)
                nc.sync.dma_start(ag_in_d[:], h2h[:])
                nc.gpsimd.collective_compute(
                    "AllGather", ALU.bypass, ins=[ag_in_d[:]],
                    outs=[ag_out_d[:]], replica_groups=rg_pair)
            nc.sync.dma_start(xmidT[:], x_mid[:])
        omypool.release()
        xopool.release()

        # ========== phase 4: MLP (pair TP-2 over inter) + chunked RS ==========
        with tc.tile_pool(name="h2c_pool", bufs=1) as h2cp, \
             tc.tile_pool(name="m_pool", bufs=1) as mp, \
             tc.tile_pool(name="wgu_pool", bufs=4) as wgup, \
             tc.tile_pool(name="wd_pool", bufs=3) as wdp, \
             tc.tile_pool(name="p7ps", bufs=2, space="PSUM") as p7ps, \
             tc.tile_pool(name="p7dps", bufs=2, space="PSUM") as p7dps, \
             tc.tile_pool(name="p7sb", bufs=4) as p7sb:
            # pair token order: [lo(2j), lo(2j+1), hi(15-2j), hi(14-2j)]
            h2c = h2cp.tile([P, HT, PTOK], FP8)
            for r in range(2):
                nc.scalar.dma_start(h2c[:, :, r * P:(r + 1) * P], ag_lo_out[r])
            m_all = mp.tile([P, IT, PTOK], BF16)
            # gate/up over lo tokens (cols 0:256) then hi tokens (cols 256:512)
            for tpart in range(2):
                cols = slice(tpart * TOK, (tpart + 1) * TOK)
                if tpart == 1:
                    for r in range(2):
                        nc.scalar.dma_start(
                            h2c[:, :, TOK + r * P:TOK + (r + 1) * P], ag_hi_out[r])
                for f in range(IT):
                    wtg = wgup.tile([P, HT2, 2, P], FP8, name="wtg")
                    nc.scalar.dma_start(wtg[:], wg_in[f])
                    gps = p7ps.tile([P, TOK], FP32, name="gps")
                    for b in range(HT2):
                        nc.tensor.matmul(gps[:], wtg[:, b, :, :],
                                         h2c[:, 2 * b:2 * b + 2, cols],
                                         start=(b == 0), stop=(b == HT2 - 1),
                                         perf_mode=DR)
                    wtu = wgup.tile([P, HT2, 2, P], FP8, name="wtu")
                    nc.scalar.dma_start(wtu[:], wu_in[f])
                    ups = p7ps.tile([P, TOK], FP32, name="ups")
                    for b in range(HT2):
                        nc.tensor.matmul(ups[:], wtu[:, b, :, :],
                                         h2c[:, 2 * b:2 * b + 2, cols],
                                         start=(b == 0), stop=(b == HT2 - 1),
                                         perf_mode=DR)
                    gr = p7sb.tile([P, TOK], BF16, name="gr")
                    nc.scalar.activation(gr[:], gps[:], AF.Relu,
                                         scale=ag[:, f:f + 1])
                    g2 = p7sb.tile([P, TOK], BF16, name="g2")
                    nc.scalar.activation(g2[:], gr[:], AF.Square)
                    nc.vector.scalar_tensor_tensor(m_all[:, f, cols], ups[:],
                                                   au[:, f:f + 1], g2[:],
                                                   ALU.mult, ALU.mult)
            # down proj (bf16) + chunked pair-RS (4 chunks of 4 f-tiles)
            for c in range(4):
                rs_in = nc.dram_tensor(f"rs_in_{c}", [4 * P, PTOK], BF16)
                rs_iv = rs_in[:].rearrange("(f p) t -> f p t", p=P)
                rs_out = nc.dram_tensor(f"rs_out_{c}", [2 * P, PTOK], BF16)
                for fi in range(4):
                    fo = 4 * c + fi
                    wtd = wdp.tile([P, IT, P], BF16, name="wtd")
                    nc.scalar.dma_start(wtd[:], wd_in[fo])
                    dps = p7dps.tile([P, PTOK], FP32, name="dps")
                    for it in range(IT):
                        nc.tensor.matmul(dps[:], wtd[:, it, :], m_all[:, it, :],
                                         start=(it == 0), stop=(it == IT - 1))
                    dn = p7sb.tile([P, PTOK], BF16, name="dn")
                    nc.scalar.activation(dn[:], dps[:], AF.Copy,
                                         scale=ad[:, fo:fo + 1])
                    nc.sync.dma_start(rs_iv[fo % 4], dn[:])
                nc.gpsimd.collective_compute(
                    "ReduceScatter", ALU.add, ins=[rs_in[:]],
                    outs=[rs_out[:]], replica_groups=rg_pair)
                nc.sync.dma_start(outD[c], rs_out[:])
            _ = wop
        midpool.release()
        const.release()

    nc.finalize()
    return nc


def _ternary(w, fold_row=None):
    """Quantize [O, Hin] fp32 -> (ternary fp32 {-1,0,1}, absmean [O])."""
    w = np.asarray(w, dtype=np.float32)
    am = np.mean(np.abs(w), axis=1)
    t = np.sign(w) * (np.abs(w) > ALPHA * am[:, None]).astype(np.float32)
    if fold_row is not None:
        t = t * fold_row[None, :]
    return t, am


def _wlhsT(tern, n_f):
    """ternary [O, Hin] -> bf16 lhsT layout [f, p, kt, c]."""
    o, hin = tern.shape
    kt = hin // P
    assert n_f * P == o
    wT = np.ascontiguousarray(tern.T)  # [Hin, O]
    return np.ascontiguousarray(
        wT.reshape(kt, P, n_f, P).transpose(2, 1, 0, 3)).astype(BF)


def _wlhsT_dr(tern, n_f):
    """ternary [O, Hin] -> fp8 DoubleRow lhsT layout [p, f, b, i, m]:
    w[p, f, b, i, m] = ternT[128*(2b+i)+p, 128*f+m]."""
    o, hin = tern.shape
    b2 = hin // (2 * P)
    assert n_f * P == o
    wT = np.ascontiguousarray(tern.T)  # [Hin, O]
    return np.ascontiguousarray(
        wT.reshape(b2, 2, P, n_f, P).transpose(2, 3, 0, 1, 4)).astype(F8)


def _wd_layout(td_slice):
    """[H, I_loc] -> bf16 [fo, p, it, m]: wd[fo, p, it, m] =
    td_slice[128*fo+m, 128*it+p]."""
    hin, iloc = td_slice.shape
    assert hin == H and iloc == I_LOC
    wT = np.ascontiguousarray(td_slice.T)  # [I_loc, H]
    return np.ascontiguousarray(
        wT.reshape(IT, P, HT, P).transpose(2, 1, 0, 3)).astype(BF)


def _scale_tiles(a):
    """[O] -> [P, O//P] with column f = features f*128..f*128+127."""
    return np.ascontiguousarray(a.reshape(-1, P).T).astype(np.float32)


def _pcol(x2d):
    """[K, T] -> [P, K//P, T] (partition-major for direct DMA)."""
    k, t = x2d.shape
    return np.ascontiguousarray(
        x2d.reshape(k // P, P, t).transpose(1, 0, 2)).astype(np.float32)


def kernel(x, cos, sin, wq, wk, wv, wo, wg, wu, wd, ln1_w, ln2_w):
    x = np.asarray(x, dtype=np.float32)
    b, s, hdim = x.shape
    assert (b, s, hdim) == (1, S, H)

    if "nc" not in _CACHE:
        _CACHE["nc"] = _build_program()
    nc = _CACHE["nc"]

    ln1 = np.asarray(ln1_w, dtype=np.float32)
    ln2 = np.asarray(ln2_w, dtype=np.float32)

    tq, amq = _ternary(wq, fold_row=ln1)
    tk, amk = _ternary(wk, fold_row=ln1)
    tv, amv = _ternary(wv, fold_row=ln1)
    to, amo = _ternary(wo)
    tg, amg = _ternary(wg, fold_row=ln2)
    tu, amu = _ternary(wu, fold_row=ln2)
    td, amd = _ternary(wd)

    wq_h = _wlhsT_dr(tq, NH)          # [P, 16, 8, 2, P]
    wk_h = _wlhsT_dr(tk, NKV)         # [P, 4, 8, 2, P]
    wv_h = _wlhsT_dr(tv, NKV)
    wo_h = _wlhsT(to, HT)             # [16, P, 16, P] bf16

    aq_h = _scale_tiles(amq / np.sqrt(np.float32(D)))
    ak_h = _scale_tiles(amk)
    av_h = _scale_tiles(amv)
    ao_h = _scale_tiles(amo)
    ag_h = _scale_tiles(amg)          # [P, 64]
    au_h = _scale_tiles(amu)
    ad_h = _scale_tiles(amd)          # [P, 16]

    x2 = x[0]
    xT = np.ascontiguousarray(x2.T)
    xT_f = _pcol(xT)
    cosT = np.ascontiguousarray(np.asarray(cos, np.float32)[0, 0].T).astype(BF)
    sinT = np.ascontiguousarray(np.asarray(sin, np.float32)[0, 0].T).astype(BF)

    R = np.zeros((P, P), np.float32)
    for m in range(64):
        R[m, m + 64] = -1.0
        R[m + 64, m] = 1.0
    rT_h = np.ascontiguousarray(R.T).astype(BF)
    ones_f = np.ones((P, P), np.float32)
    ones_b = np.ones((P, 1), np.float32).astype(BF)
    invh_b = np.full((P, 1), 1.0 / H, np.float32).astype(BF)
    triu = np.triu(np.ones((P, P), np.float32))
    tril2_h = np.ascontiguousarray(np.concatenate([triu, triu], axis=1)).astype(BF)
    iden_h = np.eye(P, dtype=np.float32).astype(BF)

    in_maps = []
    for i in range(NC):
        blo, bhi = i, 15 - i
        own_cols = np.r_[blo * P:(blo + 1) * P, bhi * P:(bhi + 1) * P]
        kvh = i // 2
        par = i % 2
        isl = slice(par * IT, (par + 1) * IT)       # inter tile slice (TP-2)
        irow = slice(par * I_LOC, (par + 1) * I_LOC)
        in_maps.append({
            "xT_f": xT_f.astype(BF),
            "xT_own": _pcol(xT[:, own_cols]),
            "cos_f": cosT, "sin_f": sinT,
            "wq": np.ascontiguousarray(wq_h[:, 2 * i:2 * i + 2]),
            "wk": np.ascontiguousarray(wk_h[:, kvh]),
            "wv": np.ascontiguousarray(wv_h[:, kvh]),
            "wo": wo_h,
            "wg": np.ascontiguousarray(
                _wlhsT_dr(tg[irow], IT).transpose(1, 0, 2, 3, 4)),
            "wu": np.ascontiguousarray(
                _wlhsT_dr(tu[irow], IT).transpose(1, 0, 2, 3, 4)),
            "wd": _wd_layout(td[:, irow]),
            "aq": np.ascontiguousarray(aq_h[:, 2 * i:2 * i + 2]),
            "ak": np.ascontiguousarray(ak_h[:, kvh:kvh + 1]),
            "av": np.ascontiguousarray(av_h[:, kvh:kvh + 1]),
            "ao": ao_h,
            "ag": np.ascontiguousarray(ag_h[:, isl]),
            "au": np.ascontiguousarray(au_h[:, isl]),
            "ad": ad_h,
            "rT": rT_h, "tril2": tril2_h, "iden": iden_h,
            "ones_f": ones_f, "ones_b": ones_b, "invh_b": invh_b,
        })

    res = run_bass_kernel_spmd(nc, in_maps, list(range(NC)))
    _CACHE["last_result"] = res

    # ---- host-side unshard: xmid residual + pair-RS output assembly ----
    out_T = np.zeros((H, S), np.float64)
    for i in range(NC):
        blo, bhi = i, 15 - i
        xm = res.results[i]["xmidT"].astype(np.float64)      # [P, HT, 256]
        xm = xm.transpose(1, 0, 2).reshape(H, TOK)
        out_T[:, blo * P:(blo + 1) * P] += xm[:, 0:P]
        out_T[:, bhi * P:(bhi + 1) * P] += xm[:, P:TOK]
    for j in range(NC // 2):
        # pair token order: [blk 2j, blk 2j+1, blk 15-2j, blk 14-2j]
        tok_cols = np.r_[(2 * j) * P:(2 * j + 1) * P,
                         (2 * j + 1) * P:(2 * j + 2) * P,
                         (15 - 2 * j) * P:(16 - 2 * j) * P,
                         (14 - 2 * j) * P:(15 - 2 * j) * P]
        for par in range(2):
            od = res.results[2 * j + par]["outD"].astype(np.float64)  # [4,256,512]
            for c in range(4):
                rows = slice(512 * c + 256 * par, 512 * c + 256 * (par + 1))
                out_T[rows][:, tok_cols] += od[c]
    return np.ascontiguousarray(out_T.T).reshape(1, S, H).astype(np.float32)


if __name__ == "__main__":
    nc = _build_program()
    print("build OK; instructions:",
          sum(len(b.instructions) for f in nc.m.functions for b in f.blocks))


# revision 15
# speedup vs baseline: 1.4473x; 1.1831x over previous
"""BitNet transformer block on 8 Trainium2 NeuronCores (Bass/Tile SPMD).

v2: fold-balanced head-parallel attention (core i owns heads {2i,2i+1},
query blocks fold-paired; A2A to token-parallel), then pair-wise TP-2
MLP (cores {2j,2j+1} split INTER 4096/4096 over their 512 tokens) with
fp8e4m3 DoubleRow matmuls for q/k/v/gate/up (ternary weights are exact
in fp8), bf16 down-proj, and chunked pair ReduceScatter in bf16.
The o_proj residual x_mid is returned per-core and added on the host
during unshard assembly (as in v1).
"""

import sys

import numpy as np

try:
    import concourse.bass as bass  # noqa: F401
except Exception:  # pragma: no cover
    sys.path.insert(0, "/opt/trn_rl_repo")

import ml_dtypes
import concourse.bass as bass
import concourse.mybir as mybir
import concourse.tile as tile
from concourse import bacc
from concourse.bass_utils import run_bass_kernel_spmd

FP32 = mybir.dt.float32
BF16 = mybir.dt.bfloat16
FP8 = mybir.dt.float8e4
BF = ml_dtypes.bfloat16
F8 = ml_dtypes.float8_e4m3

ALPHA = 0.7
EPS = 1e-5
NH = 16          # query heads
NKV = 4          # kv heads
D = 128          # head dim
H = 2048         # hidden
I_TOT = 8192     # mlp intermediate
S = 2048         # sequence
NC = 8           # cores
P = 128
HT = H // P      # 16 hidden tiles
HT2 = HT // 2    # 8 hidden tile-pairs (fp8 DoubleRow)
B = S // P       # 16 token blocks
I_LOC = I_TOT // 2    # 4096 intermediate per core (TP-2)
IT = I_LOC // P       # 32 inter tiles per core
TOK = 256             # tokens owned per core (2 blocks)
PTOK = 512            # tokens owned per pair
DR = mybir.MatmulPerfMode.DoubleRow

_CACHE = {}


def _build_program():
    nc = bacc.Bacc("TRN2", target_bir_lowering=False, debug=False, num_devices=NC)
    AF = mybir.ActivationFunctionType
    ALU = mybir.AluOpType
    rg_all = [list(range(NC))]
    rg_pair = [[2 * j, 2 * j + 1] for j in range(NC // 2)]

    # ---------------- inputs ----------------
    def dram_in(name, shape, dt=FP32):
        return nc.dram_tensor(name, shape, dt, kind="ExternalInput")

    xT_f = dram_in("xT_f", [P, HT, S], BF16)          # bf16 x^T ALL tokens (ln1 only)
    xT_own = dram_in("xT_own", [P, HT, TOK])          # fp32 x^T own cols (residual)
    cos_f = dram_in("cos_f", [P, S], BF16)
    sin_f = dram_in("sin_f", [P, S], BF16)
    wq_in = dram_in("wq", [P, 2, HT2, 2, P], FP8)     # my 2 heads [p,f,b,i,m]
    wk_in = dram_in("wk", [P, HT2, 2, P], FP8)        # my kv head
    wv_in = dram_in("wv", [P, HT2, 2, P], FP8)
    wo_in = dram_in("wo", [HT, P, HT, P], BF16)
    wg_in = dram_in("wg", [IT, P, HT2, 2, P], FP8)    # [f,p,b,i,m]
    wu_in = dram_in("wu", [IT, P, HT2, 2, P], FP8)
    wd_in = dram_in("wd", [HT, P, IT, P], BF16)       # [fo,p,it,m]
    aq_in = dram_in("aq", [P, 2])
    ak_in = dram_in("ak", [P, 1])
    av_in = dram_in("av", [P, 1])
    ao_in = dram_in("ao", [P, HT])
    ag_in = dram_in("ag", [P, IT])
    au_in = dram_in("au", [P, IT])
    ad_in = dram_in("ad", [P, HT])
    rT_in = dram_in("rT", [P, P], BF16)               # rope rotate-half perm^T
    tril_in = dram_in("tril2", [P, TOK], BF16)        # [k, q] keep k<=q, both heads
    iden_in = dram_in("iden", [P, P], BF16)           # identity for PE transpose
    ones_f_in = dram_in("ones_f", [P, P])             # fp32 ones
    ones_b_in = dram_in("ones_b", [P, 1], BF16)       # bf16 ones column
    invh_b_in = dram_in("invh_b", [P, 1], BF16)       # bf16 1/H column

    xmidT = nc.dram_tensor("xmidT", [P, HT, TOK], FP32, kind="ExternalOutput")
    outD = nc.dram_tensor("outD", [4, TOK, PTOK], BF16, kind="ExternalOutput")

    a2a_lo_in = nc.dram_tensor("a2a_lo_in", [NC, P, 2, P], BF16)
    a2a_lo_out = nc.dram_tensor("a2a_lo_out", [NC, P, 2, P], BF16)
    a2a_hi_in = nc.dram_tensor("a2a_hi_in", [NC, P, 2, P], BF16)
    a2a_hi_out = nc.dram_tensor("a2a_hi_out", [NC, P, 2, P], BF16)
    agx_in = nc.dram_tensor("agx_in", [P, HT, TOK], FP8)
    agx_out = nc.dram_tensor("agx_out", [2, P, HT, TOK], FP8)

    with tile.TileContext(nc) as tc:
        const = tc.alloc_tile_pool(name="const", bufs=1)
        ones_f = const.tile([P, P], FP32)
        ones_b = const.tile([P, 1], BF16)
        invh_b = const.tile([P, 1], BF16)
        rT = const.tile([P, P], BF16)
        iden = const.tile([P, P], BF16)
        tril2 = const.tile([P, TOK], BF16)
        aq = const.tile([P, 2], FP32)
        ak = const.tile([P, 1], FP32)
        av = const.tile([P, 1], FP32)
        ao = const.tile([P, HT], FP32)
        ag = const.tile([P, IT], FP32)
        au = const.tile([P, IT], FP32)
        ad = const.tile([P, HT], FP32)
        for dst, src in [(ones_f, ones_f_in), (ones_b, ones_b_in),
                         (invh_b, invh_b_in), (rT, rT_in),
                         (iden, iden_in), (tril2, tril_in),
                         (aq, aq_in), (ak, ak_in), (av, av_in), (ao, ao_in),
                         (ag, ag_in), (au, au_in), (ad, ad_in)]:
            nc.sync.dma_start(dst[:], src[:])

        midpool = tc.alloc_tile_pool(name="midpool", bufs=1)
        x_mid = midpool.tile([P, HT, TOK], FP32)
        xopool = tc.alloc_tile_pool(name="xopool", bufs=1)
        xo = xopool.tile([P, HT, TOK], FP32)
        omypool = tc.alloc_tile_pool(name="omypool", bufs=1)
        o_my = omypool.tile([P, HT, TOK], BF16)      # post-A2A: 16 heads x my toks
        qkvpool = tc.alloc_tile_pool(name="qkvpool", bufs=1)
        q_my = qkvpool.tile([P, 2, S], BF16)         # my 2 heads, all tokens
        k_my = qkvpool.tile([P, B, P], BF16)         # my kv head [d, blk, tok]
        v_my = qkvpool.tile([P, B, P], BF16)         # my kv head [tok, blk, d]

        def rmsnorm_t(src3d, out3d, nt, psp, tmp, odt):
            """[P,HT,nt] -> rmsnorm, partition reduce via (1/H)-column matmul,
            then broadcast-matmul BEFORE reciprocal so DVE runs on 128 lanes."""
            ssq = psp.tile([1, 512], FP32, name="ssq")[:, :nt]
            for kt in range(HT):
                sqv = tmp.tile([P, 512], BF16, name="sqv")[:, :nt]
                nc.vector.tensor_mul(sqv[:], src3d[:, kt, :], src3d[:, kt, :])
                nc.tensor.matmul(ssq[:], invh_b[:], sqv[:],
                                 start=(kt == 0), stop=(kt == HT - 1))
            ssb = tmp.tile([1, 512], FP32, name="ssb")[:, :nt]
            nc.scalar.activation(ssb[:], ssq[:], AF.Copy)
            msb = psp.tile([P, 512], FP32, name="msb")[:, :nt]
            nc.tensor.matmul(msb[:], ones_f[0:1, :], ssb[:], start=True, stop=True)
            rec = tmp.tile([P, 512], FP32, name="rec")[:, :nt]
            nc.vector.reciprocal_approx_fast(rec[:], msb[:])
            rsq = tmp.tile([P, 512], BF16, name="rsq")[:, :nt]
            nc.scalar.activation(rsq[:], rec[:], AF.Sqrt)
            for kt in range(HT):
                nc.vector.tensor_mul(out3d[:, kt, :], src3d[:, kt, :], rsq[:])
            _ = odt

        # ====== phase 1: ln1 (all tokens, chunked) + q/k/v TP projections ======
        CH4 = 512
        with tc.tile_pool(name="xc_pool", bufs=2) as xcp, \
             tc.tile_pool(name="hc_pool", bufs=2) as hcp, \
             tc.tile_pool(name="p1sb", bufs=2) as p1sb, \
             tc.tile_pool(name="p1ps", bufs=1, space="PSUM") as p1ps, \
             tc.tile_pool(name="p2ps", bufs=2, space="PSUM") as p2ps, \
             tc.tile_pool(name="rot_ps", bufs=2, space="PSUM") as rot_ps, \
             tc.tile_pool(name="vt_ps", bufs=2, space="PSUM") as vt_ps, \
             tc.tile_pool(name="p2sb", bufs=2) as p2sb, \
             tc.tile_pool(name="cs_pool", bufs=2) as csp, \
             tc.tile_pool(name="wres", bufs=1) as wres:
            wq_sb = wres.tile([P, 2, HT2, 2, P], FP8)
            nc.sync.dma_start(wq_sb[:], wq_in[:])
            wk_sb = wres.tile([P, HT2, 2, P], FP8)
            nc.sync.dma_start(wk_sb[:], wk_in[:])
            wv_sb = wres.tile([P, HT2, 2, P], FP8)
            nc.sync.dma_start(wv_sb[:], wv_in[:])
            for c4 in range(4):
                tsl = slice(c4 * CH4, (c4 + 1) * CH4)
                xc = xcp.tile([P, HT, CH4], BF16, name="xc")
                nc.scalar.dma_start(xc[:], xT_f[:, :, tsl])
                cfc = csp.tile([P, CH4], BF16, name="cfc")
                nc.scalar.dma_start(cfc[:], cos_f[:, tsl])
                sfc = csp.tile([P, CH4], BF16, name="sfc")
                nc.scalar.dma_start(sfc[:], sin_f[:, tsl])
                hc = hcp.tile([P, HT, CH4], FP8, name="hc")
                rmsnorm_t(xc, hc, CH4, p1ps, p1sb, FP8)
                # q: my 2 heads (fp8 DoubleRow over hidden pairs)
                for f in range(2):
                    ps = p2ps.tile([P, CH4], FP32, name="pps")
                    for b in range(HT2):
                        nc.tensor.matmul(ps[:], wq_sb[:, f, b, :, :],
                                         hc[:, 2 * b:2 * b + 2, :],
                                         start=(b == 0), stop=(b == HT2 - 1),
                                         perf_mode=DR)
                    qs = p2sb.tile([P, CH4], BF16, name="qs")
                    nc.scalar.activation(qs[:], ps[:], AF.Copy,
                                         scale=aq[:, f:f + 1])
                    rot = rot_ps.tile([P, CH4], FP32, name="rot")
                    nc.tensor.matmul(rot[:], rT[:], qs[:], start=True, stop=True)
                    t1 = p2sb.tile([P, CH4], BF16, name="t1")
                    nc.vector.tensor_mul(t1[:], rot[:], sfc[:])
                    t2 = p2sb.tile([P, CH4], BF16, name="t2")
                    nc.vector.tensor_mul(t2[:], qs[:], cfc[:])
                    nc.gpsimd.tensor_add(q_my[:, f, tsl], t1[:], t2[:])
                # k: my kv head
                ps = p2ps.tile([P, CH4], FP32, name="pps")
                for b in range(HT2):
                    nc.tensor.matmul(ps[:], wk_sb[:, b, :, :],
                                     hc[:, 2 * b:2 * b + 2, :],
                                     start=(b == 0), stop=(b == HT2 - 1),
                                     perf_mode=DR)
                ks = p2sb.tile([P, CH4], BF16, name="qs")
                nc.scalar.activation(ks[:], ps[:], AF.Copy, scale=ak[:, 0:1])
                rot = rot_ps.tile([P, CH4], FP32, name="rot")
                nc.tensor.matmul(rot[:], rT[:], ks[:], start=True, stop=True)
                t1 = p2sb.tile([P, CH4], BF16, name="t1")
                nc.vector.tensor_mul(t1[:], rot[:], sfc[:])
                t2 = p2sb.tile([P, CH4], BF16, name="t2")
                nc.vector.tensor_mul(t2[:], ks[:], cfc[:])
                nc.gpsimd.tensor_add(
                    k_my[:, 4 * c4:4 * c4 + 4, :].rearrange("p b t -> p (b t)"),
                    t1[:], t2[:])
                # v: my kv head, then PE-transpose to [tok, d]
                ps = p2ps.tile([P, CH4], FP32, name="pps")
                for b in range(HT2):
                    nc.tensor.matmul(ps[:], wv_sb[:, b, :, :],
                                     hc[:, 2 * b:2 * b + 2, :],
                                     start=(b == 0), stop=(b == HT2 - 1),
                                     perf_mode=DR)
                vtv = p2sb.tile([P, CH4], BF16, name="vtv")
                nc.scalar.activation(vtv[:], ps[:], AF.Copy, scale=av[:, 0:1])
                for j in range(4):
                    vtp = vt_ps.tile([P, P], BF16, name="vtp")
                    nc.tensor.transpose(vtp[:], vtv[:, j * P:(j + 1) * P], iden[:])
                    nc.vector.tensor_copy(v_my[:, 4 * c4 + j, :], vtp[:])

        # ============= phase 2: attention (triangle, paired heads) =============
        with tc.tile_pool(name="a_ps", bufs=3, space="PSUM") as a_ps, \
             tc.tile_pool(name="o_ps", bufs=2, space="PSUM") as o_ps, \
             tc.tile_pool(name="l_ps", bufs=2, space="PSUM") as l_ps, \
             tc.tile_pool(name="bc_ps", bufs=1, space="PSUM") as bc_ps, \
             tc.tile_pool(name="a_sb", bufs=3) as a_sb:
            for qb in range(B):
                r_dst = min(qb, 15 - qb)
                ops = o_ps.tile([P, TOK], FP32, name="ops")
                lps = l_ps.tile([1, TOK], FP32, name="lps")
                qv = q_my[:, :, qb * P:(qb + 1) * P]    # [P, 2, 128]
                for kb in range(qb + 1):
                    sps = a_ps.tile([P, TOK], FP32, name="sps")
                    nc.tensor.matmul(sps[:], k_my[:, kb, :], qv,
                                     start=True, stop=True)
                    pm = a_sb.tile([P, TOK], BF16, name="pm")
                    nc.scalar.activation(pm[:], sps[:], AF.Exp)
                    if kb == qb:
                        pmm = a_sb.tile([P, TOK], BF16, name="pmm")
                        nc.vector.tensor_mul(pmm[:], pm[:], tril2[:])
                        pm = pmm
                    nc.tensor.matmul(lps[:], ones_b[:], pm[:],
                                     start=(kb == 0), stop=(kb == qb))
                    nc.tensor.matmul(ops[:], v_my[:, kb, :], pm[:],
                                     start=(kb == 0), stop=(kb == qb))
                lsb = a_sb.tile([1, TOK], FP32, name="lsb")
                nc.scalar.activation(lsb[:], lps[:], AF.Copy)
                bca = bc_ps.tile([P, TOK], FP32, name="bca")
                nc.tensor.matmul(bca[:], ones_f[0:1, :], lsb[:], start=True, stop=True)
                linv = a_sb.tile([P, TOK], FP32, name="linv")
                nc.vector.reciprocal_approx_fast(linv[:], bca[:])
                osb = a_sb.tile([P, TOK], BF16, name="osb")
                nc.vector.tensor_mul(osb[:], ops[:], linv[:])
                dst = a2a_lo_in if qb < 8 else a2a_hi_in
                nc.sync.dma_start(
                    dst[r_dst][:],
                    osb[:].rearrange("p (h t) -> p h t", h=2))
                if qb == 7:
                    nc.gpsimd.collective_compute(
                        "AllToAll", ALU.bypass, ins=[a2a_lo_in[:]],
                        outs=[a2a_lo_out[:]], replica_groups=rg_all)
            nc.gpsimd.collective_compute(
                "AllToAll", ALU.bypass, ins=[a2a_hi_in[:]],
                outs=[a2a_hi_out[:]], replica_groups=rg_all)
        qkvpool.release()

        # ===== phase 3: o_proj + residual + ln2 (token halves) + pair-AG =====
        with tc.tile_pool(name="wo_pool", bufs=3) as wop, \
             tc.tile_pool(name="wo_res", bufs=1) as wores, \
             tc.tile_pool(name="p5ps", bufs=2, space="PSUM") as p5ps, \
             tc.tile_pool(name="p5sb", bufs=3) as p5sb:
            nc.sync.dma_start(xo[:], xT_own[:])
            wo_all = wores.tile([P, HT, HT, P], BF16)
            for f in range(HT):
                nc.scalar.dma_start(wo_all[:, f, :, :], wo_in[f])
            for j in range(NC):
                nc.sync.dma_start(o_my[:, 2 * j:2 * j + 2, 0:P], a2a_lo_out[j])
            for half in range(2):
                csl = slice(half * P, (half + 1) * P)
                if half == 1:
                    for j in range(NC):
                        nc.sync.dma_start(o_my[:, 2 * j:2 * j + 2, P:TOK],
                                          a2a_hi_out[j])
                for f in range(HT):
                    ps = p5ps.tile([P, P], FP32, name="ops5")
                    for kt in range(HT):
                        nc.tensor.matmul(ps[:], wo_all[:, f, kt, :], o_my[:, kt, csl],
                                         start=(kt == 0), stop=(kt == HT - 1))
                    nc.vector.scalar_tensor_tensor(
                        x_mid[:, f, csl], ps[:], ao[:, f:f + 1],
                        xo[:, f, csl], ALU.mult, ALU.add)
            h2h = p5sb.tile([P, HT, TOK], FP8, name="h2h", tag="h2h")
            rmsnorm_t(x_mid, h2h, TOK, p5ps, p5sb, FP8)
            nc.sync.dma_start(agx_in[:], h2h[:])
            nc.gpsimd.collective_compute(
                "AllGather", ALU.bypass, ins=[agx_in[:]],
                outs=[agx_out[:]], replica_groups=rg_pair)
            nc.scalar.dma_start(xmidT[:], x_mid[:])
        omypool.release()
        xopool.release()

        # ========== phase 4: MLP (pair TP-2 over inter) + chunked RS ==========
        with tc.tile_pool(name="h2c_pool", bufs=1) as h2cp, \
             tc.tile_pool(name="m_pool", bufs=1) as mp, \
             tc.tile_pool(name="wgu_pool", bufs=4) as wgup, \
             tc.tile_pool(name="wd_pool", bufs=3) as wdp, \
             tc.tile_pool(name="p7ps", bufs=2, space="PSUM") as p7ps, \
             tc.tile_pool(name="p7dps", bufs=2, space="PSUM") as p7dps, \
             tc.tile_pool(name="p7sb", bufs=4) as p7sb:
            # pair token order: [2j's 256 | (2j+1)'s 256]
            h2c = h2cp.tile([P, HT, PTOK], FP8)
            nc.sync.dma_start(h2c[:, :, 0:TOK], agx_out[0])
            nc.scalar.dma_start(h2c[:, :, TOK:PTOK], agx_out[1])
            m_all = mp.tile([P, IT, PTOK], BF16)
            for f in range(IT):
                wtg = wgup.tile([P, HT2, 2, P], FP8, name="wtg")
                nc.sync.dma_start(wtg[:], wg_in[f])
                gps = p7ps.tile([P, PTOK], FP32, name="gps")
                for b in range(HT2):
                    nc.tensor.matmul(gps[:], wtg[:, b, :, :],
                                     h2c[:, 2 * b:2 * b + 2, :],
                                     start=(b == 0), stop=(b == HT2 - 1),
                                     perf_mode=DR)
                wtu = wgup.tile([P, HT2, 2, P], FP8, name="wtu")
                nc.sync.dma_start(wtu[:], wu_in[f])
                ups = p7ps.tile([P, PTOK], FP32, name="ups")
                for b in range(HT2):
                    nc.tensor.matmul(ups[:], wtu[:, b, :, :],
                                     h2c[:, 2 * b:2 * b + 2, :],
                                     start=(b == 0), stop=(b == HT2 - 1),
                                     perf_mode=DR)
                gr = p7sb.tile([P, PTOK], BF16, name="gr")
                nc.vector.tensor_scalar(gr[:], gps[:], ag[:, f:f + 1], 0.0,
                                        ALU.mult, ALU.max)
                g2 = p7sb.tile([P, PTOK], BF16, name="g2")
                nc.gpsimd.tensor_mul(g2[:], gr[:], gr[:])
                nc.vector.scalar_tensor_tensor(m_all[:, f, :], ups[:],
                                               au[:, f:f + 1], g2[:],
                                               ALU.mult, ALU.mult)
            # down proj (bf16) + chunked pair-RS (4 chunks of 4 f-tiles)
            for c in range(4):
                rs_in = nc.dram_tensor(f"rs_in_{c}", [4 * P, PTOK], BF16)
                rs_iv = rs_in[:].rearrange("(f p) t -> f p t", p=P)
                rs_out = nc.dram_tensor(f"rs_out_{c}", [2 * P, PTOK], BF16)
                for fi in range(4):
                    fo = 4 * c + fi
                    wtd = wdp.tile([P, IT, P], BF16, name="wtd")
                    nc.sync.dma_start(wtd[:], wd_in[fo])
                    dps = p7dps.tile([P, PTOK], FP32, name="dps")
                    for it in range(IT):
                        nc.tensor.matmul(dps[:], wtd[:, it, :], m_all[:, it, :],
                                         start=(it == 0), stop=(it == IT - 1))
                    dn = p7sb.tile([P, PTOK], BF16, name="dn")
                    nc.scalar.activation(dn[:], dps[:], AF.Copy,
                                         scale=ad[:, fo:fo + 1])
                    nc.sync.dma_start(rs_iv[fo % 4], dn[:])
                nc.gpsimd.collective_compute(
                    "ReduceScatter", ALU.add, ins=[rs_in[:]],
                    outs=[rs_out[:]], replica_groups=rg_pair)
                nc.sync.dma_start(outD[c], rs_out[:])
            _ = wop
        midpool.release()
        const.release()

    nc.finalize()
    return nc


def _ternary(w, fold_row=None):
    """Quantize [O, Hin] fp32 -> (ternary fp32 {-1,0,1}, absmean [O])."""
    w = np.asarray(w, dtype=np.float32)
    am = np.mean(np.abs(w), axis=1)
    t = np.sign(w) * (np.abs(w) > ALPHA * am[:, None]).astype(np.float32)
    if fold_row is not None:
        t = t * fold_row[None, :]
    return t, am


def _wlhsT(tern, n_f):
    """ternary [O, Hin] -> bf16 lhsT layout [f, p, kt, c]."""
    o, hin = tern.shape
    kt = hin // P
    assert n_f * P == o
    wT = np.ascontiguousarray(tern.T)  # [Hin, O]
    return np.ascontiguousarray(
        wT.reshape(kt, P, n_f, P).transpose(2, 1, 0, 3)).astype(BF)


def _wlhsT_dr(tern, n_f):
    """ternary [O, Hin] -> fp8 DoubleRow lhsT layout [p, f, b, i, m]:
    w[p, f, b, i, m] = ternT[128*(2b+i)+p, 128*f+m]."""
    o, hin = tern.shape
    b2 = hin // (2 * P)
    assert n_f * P == o
    wT = np.ascontiguousarray(tern.T)  # [Hin, O]
    return np.ascontiguousarray(
        wT.reshape(b2, 2, P, n_f, P).transpose(2, 3, 0, 1, 4)).astype(F8)


def _wd_layout(td_slice):
    """[H, I_loc] -> bf16 [fo, p, it, m]: wd[fo, p, it, m] =
    td_slice[128*fo+m, 128*it+p]."""
    hin, iloc = td_slice.shape
    assert hin == H and iloc == I_LOC
    wT = np.ascontiguousarray(td_slice.T)  # [I_loc, H]
    return np.ascontiguousarray(
        wT.reshape(IT, P, HT, P).transpose(2, 1, 0, 3)).astype(BF)


def _scale_tiles(a):
    """[O] -> [P, O//P] with column f = features f*128..f*128+127."""
    return np.ascontiguousarray(a.reshape(-1, P).T).astype(np.float32)


def _pcol(x2d):
    """[K, T] -> [P, K//P, T] (partition-major for direct DMA)."""
    k, t = x2d.shape
    return np.ascontiguousarray(
        x2d.reshape(k // P, P, t).transpose(1, 0, 2)).astype(np.float32)


def kernel(x, cos, sin, wq, wk, wv, wo, wg, wu, wd, ln1_w, ln2_w):
    x = np.asarray(x, dtype=np.float32)
    b, s, hdim = x.shape
    assert (b, s, hdim) == (1, S, H)

    if "nc" not in _CACHE:
        _CACHE["nc"] = _build_program()
    nc = _CACHE["nc"]

    ln1 = np.asarray(ln1_w, dtype=np.float32)
    ln2 = np.asarray(ln2_w, dtype=np.float32)

    tq, amq = _ternary(wq, fold_row=ln1)
    tk, amk = _ternary(wk, fold_row=ln1)
    tv, amv = _ternary(wv, fold_row=ln1)
    to, amo = _ternary(wo)
    tg, amg = _ternary(wg, fold_row=ln2)
    tu, amu = _ternary(wu, fold_row=ln2)
    td, amd = _ternary(wd)

    wq_h = _wlhsT_dr(tq, NH)          # [P, 16, 8, 2, P]
    wk_h = _wlhsT_dr(tk, NKV)         # [P, 4, 8, 2, P]
    wv_h = _wlhsT_dr(tv, NKV)
    wo_h = _wlhsT(to, HT)             # [16, P, 16, P] bf16

    aq_h = _scale_tiles(amq / np.sqrt(np.float32(D)))
    ak_h = _scale_tiles(amk)
    av_h = _scale_tiles(amv)
    ao_h = _scale_tiles(amo)
    ag_h = _scale_tiles(amg)          # [P, 64]
    au_h = _scale_tiles(amu)
    ad_h = _scale_tiles(amd)          # [P, 16]

    x2 = x[0]
    xT = np.ascontiguousarray(x2.T)
    xT_f = _pcol(xT)
    cosT = np.ascontiguousarray(np.asarray(cos, np.float32)[0, 0].T).astype(BF)
    sinT = np.ascontiguousarray(np.asarray(sin, np.float32)[0, 0].T).astype(BF)

    R = np.zeros((P, P), np.float32)
    for m in range(64):
        R[m, m + 64] = -1.0
        R[m + 64, m] = 1.0
    rT_h = np.ascontiguousarray(R.T).astype(BF)
    ones_f = np.ones((P, P), np.float32)
    ones_b = np.ones((P, 1), np.float32).astype(BF)
    invh_b = np.full((P, 1), 1.0 / H, np.float32).astype(BF)
    triu = np.triu(np.ones((P, P), np.float32))
    tril2_h = np.ascontiguousarray(np.concatenate([triu, triu], axis=1)).astype(BF)
    iden_h = np.eye(P, dtype=np.float32).astype(BF)

    in_maps = []
    for i in range(NC):
        blo, bhi = i, 15 - i
        own_cols = np.r_[blo * P:(blo + 1) * P, bhi * P:(bhi + 1) * P]
        kvh = i // 2
        par = i % 2
        isl = slice(par * IT, (par + 1) * IT)       # inter tile slice (TP-2)
        irow = slice(par * I_LOC, (par + 1) * I_LOC)
        in_maps.append({
            "xT_f": xT_f.astype(BF),
            "xT_own": _pcol(xT[:, own_cols]),
            "cos_f": cosT, "sin_f": sinT,
            "wq": np.ascontiguousarray(wq_h[:, 2 * i:2 * i + 2]),
            "wk": np.ascontiguousarray(wk_h[:, kvh]),
            "wv": np.ascontiguousarray(wv_h[:, kvh]),
            "wo": wo_h,
            "wg": np.ascontiguousarray(
                _wlhsT_dr(tg[irow], IT).transpose(1, 0, 2, 3, 4)),
            "wu": np.ascontiguousarray(
                _wlhsT_dr(tu[irow], IT).transpose(1, 0, 2, 3, 4)),
            "wd": _wd_layout(td[:, irow]),
            "aq": np.ascontiguousarray(aq_h[:, 2 * i:2 * i + 2]),
            "ak": np.ascontiguousarray(ak_h[:, kvh:kvh + 1]),
            "av": np.ascontiguousarray(av_h[:, kvh:kvh + 1]),
            "ao": ao_h,
            "ag": np.ascontiguousarray(ag_h[:, isl]),
            "au": np.ascontiguousarray(au_h[:, isl]),
            "ad": ad_h,
            "rT": rT_h, "tril2": tril2_h, "iden": iden_h,
            "ones_f": ones_f, "ones_b": ones_b, "invh_b": invh_b,
        })

    res = run_bass_kernel_spmd(nc, in_maps, list(range(NC)))
    _CACHE["last_result"] = res

    # ---- host-side unshard: xmid residual + pair-RS output assembly ----
    out_T = np.zeros((H, S), np.float64)
    for i in range(NC):
        blo, bhi = i, 15 - i
        xm = res.results[i]["xmidT"].astype(np.float64)      # [P, HT, 256]
        xm = xm.transpose(1, 0, 2).reshape(H, TOK)
        out_T[:, blo * P:(blo + 1) * P] += xm[:, 0:P]
        out_T[:, bhi * P:(bhi + 1) * P] += xm[:, P:TOK]
    for j in range(NC // 2):
        # pair token order: [core 2j's 256 | core 2j+1's 256]
        tok_cols = np.r_[(2 * j) * P:(2 * j + 1) * P,
                         (15 - 2 * j) * P:(16 - 2 * j) * P,
                         (2 * j + 1) * P:(2 * j + 2) * P,
                         (14 - 2 * j) * P:(15 - 2 * j) * P]
        for par in range(2):
            od = res.results[2 * j + par]["outD"].astype(np.float64)  # [4,256,512]
            for c in range(4):
                rows = slice(512 * c + 256 * par, 512 * c + 256 * (par + 1))
                out_T[rows][:, tok_cols] += od[c]
    return np.ascontiguousarray(out_T.T).reshape(1, S, H).astype(np.float32)


if __name__ == "__main__":
    nc = _build_program()
    print("build OK; instructions:",
          sum(len(b.instructions) for f in nc.m.functions for b in f.blocks))
